# revision 8
# baseline (speedup 1.0000x reference)
"""Trainium2 Bass kernel for DigitConvolutionalModel.

Computes, for x [32768, 784] viewed as 28x28 images:
    feat = relu(conv3x3_valid(x))      # [B, 676]
    out  = feat @ W + b                # [B, 10]

Strategy (pure data parallel over 8 cores, 4096 rows each):
  - Host pre-casts x to bf16, chunk-major partition-contiguous and
    UNPADDED: chunk n (512 samples) is xh rows [128n,128n+128) with
    pixel tiles 0..5 (pixels 0..767) side by side (6144B/row); the
    16-pixel remainder (768..783) rides in one shared stub tensor
    xs [16, 8*512] (chunk n at cols 512n).
  - All loads are HWDGE DMAs on the sync ring, ordered so compute can
    start as early as possible: C-matrix head (tiles u=0,1), x chunk0
    tiles 0-2, C rest, x chunk0 tiles 3-5, stubs, chunks 1..7.
    Consumers absorb DMA waits via tiny "touch" matmuls so real
    instructions carry at most one sync wait (walrus limit).
  - Conv as banded matmul over 128-pixel tiles: output tile u =
    C1_u^T @ xtile_u + C2_u^T @ xtile_{u+1} (u=5: C1_5 [128,36] from
    tile5 + C2s [16,36] from the stub). 12 conv matmuls per chunk;
    relu evacuates PSUM -> SBUF bf16 alternating ScalarE/VectorE into
    statically-allocated ry tiles (48 total, no reuse hazards).
  - Linear with PE column tiling: the 676->10 matmul wastes 118/128
    PSUM partitions, so 4 chunks' linear streams run CONCURRENTLY in
    the four 32-column strips of the array (tile_position=(0,32j)),
    accumulating over the 6 feature tiles into one PSUM bank.  Bias is
    folded in as a 37th feature row of tile5 (ry5 row 36 = 1.0, wp5
    row 36 = b), so no separate bias stage.  One ScalarE copy per
    4-chunk group evacuates all four strips; SWDGE stores follow.
  - PE HAM warm-up: junk matmuls from the earliest post-preamble slot
    soak the initial DMA wait so the 2.4 GHz grant arrives close to
    when real conv work begins.  No tail junk: the walrus semaphore
    -file reset at kernel end was measured to pace identically warm
    vs cold.
  - Kernel-exit drains are spread across sync/scalar/vector/tensor
    only (GpSimd DRAIN costs ~260ns vs ~10ns elsewhere).
"""

import numpy as np

try:
    from concourse import bass, mybir
    from concourse.tile import TileContext
    from concourse.bass_utils import run_bass_kernel_spmd
except ImportError:  # path used when concourse is not already importable
    import sys

    sys.path.insert(0, "/opt/trn_rl_repo")
    from concourse import bass, mybir
    from concourse.tile import TileContext
    from concourse.bass_utils import run_bass_kernel_spmd

from concourse.vector_clock import ScopedClock


def _patched_drain_and_barrier(self, tick_clock, wait_clock):
    """Replacement for TileContext._drain_and_barrier: walrus rejects
    instructions carrying more than one sync wait, but the kernel-tail
    drain aggregates a wait per logical proc (~25 here). Emit
    single-wait drains spread across the FAST engine queues (they run
    concurrently); GpSimd drains cost ~260ns each so it is excluded."""
    nc = self.nc
    drain_inst = nc.sync.drain()
    wait_clock.add_sem_waits(
        drain_inst.ins, ScopedClock({None: tick_clock.global_clock})
    )
    si = drain_inst.ins.sync_info
    waits = list(si.on_wait or []) if si else []
    if len(waits) > 1:
        drain_inst.ins.sync_info = mybir.SyncInfo(
            on_wait=waits[:1], on_update=si.on_update
        )
        queues = [nc.sync, nc.scalar, nc.vector, nc.tensor]
        for i, w in enumerate(waits[1:]):
            eng = queues[i % len(queues)]
            extra = eng.drain()
            esi = extra.ins.sync_info
            extra.ins.sync_info = mybir.SyncInfo(
                on_wait=[w], on_update=(esi.on_update if esi else [])
            )
    nc.all_engine_barrier()
    popped = nc._tile_sem_poison_stack.pop()
    assert popped is self._sem_poison
    nc.clear_and_free_semaphores(list(self.sems.allocated().values()))
    nc.all_engine_barrier()


TileContext._drain_and_barrier = _patched_drain_and_barrier

N_CORES = 8
B = 32768
B_CORE = B // N_CORES  # 4096
N_CHUNKS = 8
CHUNK = 512  # PSUM fp32 bank limit
NT = 6  # full 128-pixel input tiles (768 pixels; 16-pixel stub separate)
NU = 6  # output tiles (5 x 128 + 36 packed valid conv outputs, 676 total)
N_JUNK_HEAD = 20  # PE warm-up matmuls before real work (~107ns each cold)

F32 = mybir.dt.float32
BF16 = mybir.dt.bfloat16
RELU = mybir.ActivationFunctionType.Relu
IDENT = mybir.ActivationFunctionType.Identity

# cpk column layout (see _build_consts)
CPK_A_COLS = 512  # C1_0|C2_0|C1_1|C2_1
C15_OFF = 512 + 768  # 1280: C1_5 [128,36]
C2S_OFF = C15_OFF + 36  # 1316: C2s [16,36] on partitions 0..15
WP_OFF = C2S_OFF + 36  # 1352: wp_0..wp_4 [128, 50]
WP5_OFF = WP_OFF + 50  # 1402: wp5 [37, 10] (row 36 = bias)
CPK_COLS = WP5_OFF + 10  # 1412

_NC_CACHE = {}


def _build_nc():
    nc = bass.Bass(
        "TRN2", target_bir_lowering=False, debug=False, num_devices=1
    )

    xh = nc.dram_tensor("xh", [N_CHUNKS * 128, NT * CHUNK], BF16, kind="ExternalInput")
    xs = nc.dram_tensor("xs", [16, N_CHUNKS * CHUNK], BF16, kind="ExternalInput")
    cpk_d = nc.dram_tensor("cpk", [128, CPK_COLS], BF16, kind="ExternalInput")
    out_t = nc.dram_tensor("out_t", [10, B_CORE], F32, kind="ExternalOutput")

    with TileContext(nc) as tc:
        with (
            tc.tile_pool(name="const", bufs=1) as cpool,
            tc.tile_pool(name="xc", bufs=1) as xpool,
            tc.tile_pool(name="ry", bufs=1) as rypool,
            tc.tile_pool(name="outT", bufs=1) as opool,
            tc.tile_pool(name="yps_a", bufs=2, space="PSUM") as ypool_a,
            tc.tile_pool(name="yps_v", bufs=2, space="PSUM") as ypool_v,
            tc.tile_pool(name="opsum", bufs=1, space="PSUM") as opsum,
        ):
            xc = [
                xpool.tile([128, NT * CHUNK], BF16, tag=f"xc{n}", name=f"xc{n}")
                for n in range(N_CHUNKS)
            ]
            xstub = xpool.tile(
                [16, N_CHUNKS * CHUNK], BF16, tag="xstub", name="xstub"
            )
            cpk_sb = cpool.tile([128, CPK_COLS], BF16, tag="cpk", name="cpk")
            # static ry tiles: no reuse -> no WAR waits anywhere
            ry = [
                [
                    rypool.tile(
                        [128, CHUNK], BF16, tag=f"ry{n}_{u}", name=f"ry{n}_{u}"
                    )
                    for u in range(NU)
                ]
                for n in range(N_CHUNKS)
            ]
            outT = [
                opool.tile([128, CHUNK], F32, tag=f"outT{s}", name=f"outT{s}")
                for s in range(2)
            ]
            junk = cpool.tile([128, 256], BF16, tag="junk", name="junk")
            ops = [
                opsum.tile([128, CHUNK], F32, tag=f"ops{s}", name=f"ops{s}")
                for s in range(2)
            ]

            # ---- loads: x on the sync HWDGE ring, constants on the
            # scalar ring so the trigger issue (~0.65us each) runs in
            # parallel and chunk-0 data lands as early as possible ----
            nc.sync.dma_start(
                xc[0][:, 0 : 3 * CHUNK], xh.ap()[0:128, 0 : 3 * CHUNK]
            )
            nc.sync.dma_start(
                xc[0][:, 3 * CHUNK :], xh.ap()[0:128, 3 * CHUNK :]
            )
            nc.sync.dma_start(xstub[:], xs.ap())
            for n in range(1, N_CHUNKS):
                nc.sync.dma_start(xc[n][:], xh.ap()[128 * n : 128 * (n + 1), :])
            nc.scalar.dma_start(cpk_sb[:, 0:CPK_A_COLS], cpk_d.ap()[:, 0:CPK_A_COLS])
            nc.scalar.dma_start(cpk_sb[:, CPK_A_COLS:], cpk_d.ap()[:, CPK_A_COLS:])

            # ---- vector-engine preamble: junk data + ones rows ----
            nc.vector.memset(junk[:], 0.0)
            # ones for the bias row: partition access must be 32-aligned,
            # so fill rows 32..63 (relu later overwrites 32..35 with real
            # features; rows 37..63 hit zero rows of wp5; row 36 = 1.0).
            for n in range(N_CHUNKS):
                nc.vector.memset(ry[n][5][32:64, :], 1.0)

            # ---- PE HAM warm-up: spam junk matmuls from the earliest
            # moment so the power grant arrives early and the initial
            # DMA wait is soaked.
            with tc.high_priority():
                for i in range(N_JUNK_HEAD):
                    nc.tensor.matmul(
                        ops[0][0:128, 0:128],
                        junk[:, 0:128],
                        junk[:, 128:256],
                        skip_group_check=True,
                    )

            warm = ops[1][0:4, 0:4]

            def touch(src):
                nc.tensor.matmul(warm, src, src, skip_group_check=True)

            c1 = lambda u: (
                cpk_sb[:, 256 * u : 256 * u + 128]
                if u < 2
                else cpk_sb[:, 512 + 256 * (u - 2) : 512 + 256 * (u - 2) + 128]
                if u < 5
                else cpk_sb[:, C15_OFF : C15_OFF + 36]
            )
            c2 = lambda u: (
                cpk_sb[:, 256 * u + 128 : 256 * u + 256]
                if u < 2
                else cpk_sb[:, 512 + 256 * (u - 2) + 128 : 512 + 256 * (u - 2) + 256]
            )
            c2s = cpk_sb[0:16, C2S_OFF : C2S_OFF + 36]
            wp = lambda u: (
                cpk_sb[:, WP_OFF + 10 * u : WP_OFF + 10 * (u + 1)]
                if u < 5
                else cpk_sb[0:37, WP5_OFF : WP5_OFF + 10]
            )

            touch(cpk_sb[:, 0:4])  # absorbs cpkA dma wait
            touch(xc[0][:, 0:4])  # absorbs x0 tiles0-2 dma wait
            cpkB_touched = False

            for n in range(N_CHUNKS):
                if n > 0:
                    touch(xc[n][:, 0:4])
                for u in range(NU):
                    if n == 0 and u == 2:
                        touch(cpk_sb[:, CPK_A_COLS : CPK_A_COLS + 4])
                        touch(xc[0][:, 3 * CHUNK : 3 * CHUNK + 4])
                        cpkB_touched = True
                    on_act = u % 2 == 0
                    yps = (ypool_a if on_act else ypool_v).tile(
                        [128, CHUNK], F32, tag="yps"
                    )
                    if u < 5:
                        nc.tensor.matmul(
                            yps[:],
                            c1(u),
                            xc[n][:, CHUNK * u : CHUNK * (u + 1)],
                            start=True,
                            stop=False,
                        )
                        nc.tensor.matmul(
                            yps[:],
                            c2(u),
                            xc[n][:, CHUNK * (u + 1) : CHUNK * (u + 2)],
                            start=False,
                            stop=True,
                        )
                        nrow = 128
                    else:
                        nc.tensor.matmul(
                            yps[0:36, :],
                            c1(5),
                            xc[n][:, 5 * CHUNK : 6 * CHUNK],
                            start=True,
                            stop=False,
                        )
                        nc.tensor.matmul(
                            yps[0:36, :],
                            c2s,
                            xstub[:, CHUNK * n : CHUNK * (n + 1)],
                            start=False,
                            stop=True,
                        )
                        nrow = 36
                    if on_act:
                        nc.scalar.activation(
                            ry[n][u][0:nrow, :], yps[0:nrow, :], RELU
                        )
                    else:
                        nc.vector.tensor_relu(ry[n][u][0:nrow, :], yps[0:nrow, :])

                # ---- linear for a 4-chunk group, col-tiled: chunk
                # 4s+j accumulates in PSUM partitions 32j..32j+9 ----
                if n % 4 == 3:
                    s = n // 4
                    for u in range(NU):
                        k = 128 if u < 5 else 37
                        for j in range(4):
                            nc.tensor.matmul(
                                ops[s][32 * j : 32 * j + 10, :],
                                wp(u),
                                ry[4 * s + j][u][0:k, :],
                                start=(u == 0),
                                stop=(u == NU - 1),
                                tile_position=(0, 32 * j),
                                skip_group_check=True,
                            )
                    nc.scalar.activation(
                        outT[s][0:106, :], ops[s][0:106, :], IDENT
                    )
                    # stores on the scalar HWDGE ring: the trigger sits
                    # after the copy on the SAME engine queue, so program
                    # order covers the data dep (the Activation self-wait
                    # is stripped below) and each trigger carries at most
                    # the ring lane wait.
                    for j in range(4):
                        c = 4 * s + j
                        nc.scalar.dma_start(
                            out_t.ap()[:, CHUNK * c : CHUNK * (c + 1)],
                            outT[s][32 * j : 32 * j + 10, :],
                        )
            assert cpkB_touched

    _strip_self_waits(nc)
    return nc


_ENGINE_SEM_PREFIX = {
    mybir.EngineType.PE: "PE_",
    mybir.EngineType.Activation: "Activation_",
    mybir.EngineType.DVE: "DVE_",
    mybir.EngineType.Pool: "Pool_",
    mybir.EngineType.SP: "SP_",
}


def _strip_self_waits(nc):
    """Drop semaphore waits an instruction holds on its OWN engine's
    completion counter. Engines execute their queue strictly in order, so
    a wait on the own-engine sem at a value covered by program order is
    redundant — but Tile still emits it, and walrus rejects compute
    instructions carrying more than one sync wait."""
    for fn in nc.m.functions:
        for blk in fn.blocks:
            for inst in blk.instructions:
                tn = type(inst).__name__
                if tn in ("InstDrain", "InstEventSemaphore"):
                    continue
                if tn == "InstDMACopy" and inst.engine not in (
                    mybir.EngineType.SP,
                    mybir.EngineType.Activation,
                ):
                    continue
                si = inst.sync_info
                if si is None or not si.on_wait or len(si.on_wait) < 2:
                    continue
                pref = _ENGINE_SEM_PREFIX.get(inst.engine)
                if pref is None:
                    continue
                kept = [w for w in si.on_wait if not w.ant_name.startswith(pref)]
                if len(kept) != len(si.on_wait):
                    inst.sync_info = mybir.SyncInfo(
                        on_wait=kept, on_update=si.on_update
                    )


def _build_consts(conv_w, W, b):
    conv_w = np.asarray(conv_w, np.float32)
    W = np.asarray(W, np.float32)
    b = np.asarray(b, np.float32)

    # Banded conv matrices for the unpadded natural-order pixel tiling.
    # Valid conv outputs packed densely: k = 26*i + j <-> pixel
    # o = 28*i + j; output tile u = k//128 (tile 5 has 36 outputs);
    # input tile t = p//128 for p < 768, stub rows for p >= 768.
    c1 = np.zeros((NU, 128, 128), np.float32)
    c2 = np.zeros((5, 128, 128), np.float32)
    c2s = np.zeros((16, 36), np.float32)
    for k in range(676):
        u, m = divmod(k, 128)
        i, j = divmod(k, 26)
        o = 28 * i + j
        for di in range(3):
            for dj in range(3):
                p = o + 28 * di + dj
                w = conv_w[di, dj]
                if p >= 768:
                    assert u == 5
                    c2s[p - 768, m] = w
                else:
                    t, r = divmod(p, 128)
                    if t == u:
                        c1[u, r, m] = w
                    else:
                        assert t == u + 1
                        c2[u, r, m] = w

    cpk = np.zeros((128, CPK_COLS), np.float32)
    for u in range(2):
        cpk[:, 256 * u : 256 * u + 128] = c1[u]
        cpk[:, 256 * u + 128 : 256 * u + 256] = c2[u]
    for u in range(2, 5):
        off = 512 + 256 * (u - 2)
        cpk[:, off : off + 128] = c1[u]
        cpk[:, off + 128 : off + 256] = c2[u]
    cpk[:, C15_OFF : C15_OFF + 36] = c1[5][:, 0:36]
    cpk[0:16, C2S_OFF : C2S_OFF + 36] = c2s
    for u in range(5):
        cpk[:, WP_OFF + 10 * u : WP_OFF + 10 * (u + 1)] = W[128 * u : 128 * (u + 1)]
    cpk[0:36, WP5_OFF : WP5_OFF + 10] = W[640:676]
    cpk[36, WP5_OFF : WP5_OFF + 10] = b  # bias via the ones-row of ry5

    import ml_dtypes

    return cpk.astype(ml_dtypes.bfloat16)


def _pack_x(x):
    """[32768, 784] fp32 -> per-core ([1024, 3072], [16, 4096]) bf16:
    chunk-major partition-contiguous, unpadded.  xh row 128n+p, col
    512t+s <- chunk n, pixel 128t+p, sample s (t<6); xs row p, col
    512n+s <- chunk n, pixel 768+p, sample s."""
    import ml_dtypes

    xb = np.ascontiguousarray(x.astype(ml_dtypes.bfloat16))
    shards = []
    for c in range(N_CORES):
        shard = xb[c * B_CORE : (c + 1) * B_CORE]  # [4096, 784]
        xp = shard.reshape(N_CHUNKS, CHUNK, 784).transpose(0, 2, 1)  # [8,784,512]
        main = np.ascontiguousarray(
            xp[:, 0:768, :].reshape(N_CHUNKS, NT, 128, CHUNK).transpose(0, 2, 1, 3)
        ).reshape(N_CHUNKS * 128, NT * CHUNK)
        stub = np.ascontiguousarray(
            xp[:, 768:784, :].transpose(1, 0, 2)
        ).reshape(16, N_CHUNKS * CHUNK)
        shards.append((main, stub))
    return shards


def _run(inputs, trace=False):
    x = np.asarray(inputs["x"], np.float32)
    conv_w = inputs["conv_w"]
    W = inputs["W"]
    b = inputs["b"]

    if "nc" not in _NC_CACHE:
        _NC_CACHE["nc"] = _build_nc()
    nc = _NC_CACHE["nc"]

    cpk = _build_consts(conv_w, W, b)
    shards = _pack_x(x)

    in_maps = [
        {"xh": shards[c][0], "xs": shards[c][1], "cpk": cpk}
        for c in range(N_CORES)
    ]

    res = run_bass_kernel_spmd(
        nc, in_maps, core_ids=list(range(N_CORES)), trace=trace
    )
    out = np.concatenate(
        [np.asarray(res.results[c]["out_t"]).T for c in range(N_CORES)], axis=0
    )
    return out, res


def kernel(**inputs) -> np.ndarray:
    return _run(inputs, trace=False)[0]
